# revision 7
# baseline (speedup 1.0000x reference)
"""Contrastive loss (supervised NT-Xent style) on 8 Trainium2 NeuronCores.

Math (reference semantics):
    xn = logits / max(||logits||, 1e-8); s = xn @ xn.T; u = s / T (T=0.5)
    For row i with same-label set S_i (excl. diag), D_i = sum_{j not in S_i} exp(u_ij):
        loss*2n = sum_i sum_{j in S_i} [ log(exp(u_ij) + D_i) - u_ij ]
    The -u_ij part is computed globally via symmetry:
        sum_{i,j same-label incl diag} u_ij = 2 * sum_g ||G_g||^2,  G_g = sum_{j in seg g} xn_j
    Diagonal terms are removed analytically (u_ii = 2, e_ii = exp(2)).

Sharding: rows sorted by label on host (loss is permutation invariant);
each core owns a 1024-row strip, holds a replicated normalized xn^T, computes
its [1024, 8192] similarity strip blockwise, returns per-row partial sums.
Per-core variation (row strip, label-segment windows) is carried entirely by
input tensors + register-dynamic slices so all cores share one SPMD program.
"""

import os
import sys

for _p in ("/opt/trn_rl_repo", "/root/.axon_site/_ro/trn_rl_repo"):
    if os.path.isdir(_p) and _p not in sys.path:
        sys.path.append(_p)

import numpy as np
import ml_dtypes

TRACE = False          # test harness sets True to capture an NTFF profile
LAST_EXEC_NS = None    # filled when TRACE
LAST_RESULTS = None

N = 8192
DF = 256
NCORES = 8
RPC = N // NCORES       # rows per core
NB = RPC // 128         # 128-row blocks per core
CH = 512                # psum chunk (free dim per matmul)
NCH = N // CH
T_SCALE = 2.0           # 1 / temperature
E2 = float(np.exp(2.0))


def _emit(nc, W_CH, WIN, seg_off, seg_w):
    import concourse.bass as bass
    import concourse.mybir as mybir
    import concourse.tile as tile
    from contextlib import ExitStack

    dt = mybir.dt
    AF = mybir.ActivationFunctionType
    ALU = mybir.AluOpType
    X = mybir.AxisListType.X
    WF = W_CH * CH
    n_segs = len(seg_off)

    logits_d = nc.dram_tensor("logits", [N, DF], dt.float32, kind="ExternalInput").ap()
    mine_d = nc.dram_tensor("mine", [RPC, DF], dt.float32, kind="ExternalInput").ap()
    bounds_d = nc.dram_tensor("bounds", [RPC, 2], dt.float32, kind="ExternalInput").ap()
    ident_d = nc.dram_tensor("ident", [128, 128], dt.bfloat16, kind="ExternalInput").ap()
    acc_d = nc.dram_tensor("acc", [128, 1], dt.float32, kind="ExternalOutput").ap()
    gvec_d = nc.dram_tensor("gvec", [n_segs, 1], dt.float32, kind="ExternalOutput").ap()

    with tile.TileContext(nc) as tc, ExitStack() as ctx:
        def pool(name, bufs, space="SBUF"):
            return ctx.enter_context(tc.tile_pool(name=name, bufs=bufs, space=space))

        const = pool("const", 1)
        xp = pool("x", 3)
        sqp = pool("sq", 2)
        nrm = pool("nrm", 3)
        xnp = pool("xn", 3)
        tpp = pool("tp_psum", 2, space="PSUM")
        mmp = pool("mm_psum", 3, space="PSUM")
        gp = pool("g_psum", 1, space="PSUM")
        ep = pool("e", 2)
        rsp = pool("rs", 2)
        bp = pool("bnd", 2)
        mkp = pool("mask", 2)
        jkp = pool("junk", 2)
        lgp = pool("lg", 2)
        sm = pool("small", 3)

        xnT = [const.tile([128, N], dt.bfloat16, tag=f"xnT{t}", name=f"xnT{t}") for t in range(2)]
        mnT = [const.tile([128, RPC], dt.bfloat16, tag=f"mnT{t}", name=f"mnT{t}") for t in range(2)]
        iota_i = const.tile([128, WF], dt.int32, tag="iota_i", name="iota_i")
        iota_t = const.tile([128, WF], dt.float32, tag="iota", name="iota")
        ident_sb = const.tile([128, 128], dt.bfloat16, tag="ident", name="ident")
        acc_t = const.tile([128, 1], dt.float32, tag="acc", name="acc")
        ones_t = const.tile([128, 1], dt.float32, tag="ones", name="ones")
        G = [const.tile([128, n_segs], dt.float32, tag=f"G{t}", name=f"G{t}") for t in range(2)]
        G2 = [const.tile([128, n_segs], dt.float32, tag=f"G2{t}", name=f"G2{t}") for t in range(2)]
        gsb = const.tile([n_segs, 1], dt.float32, tag="gsb", name="gsb")
        e2c = const.tile([128, 1], dt.float32, tag="e2c", name="e2c")

        nc.sync.dma_start(ident_sb[:], ident_d[:])
        nc.gpsimd.iota(iota_i[:], [[1, WF]], base=0, channel_multiplier=0)
        nc.vector.tensor_copy(iota_t[:], iota_i[:])
        nc.vector.memset(acc_t[:], 0.0)
        nc.vector.memset(ones_t[:], 1.0)
        nc.vector.memset(e2c[:], E2)

        def norm_tiles(src_ap, n_tiles, dstT):
            for ti in range(n_tiles):
                x = xp.tile([128, DF], dt.float32, tag="x", name="x")
                nc.sync.dma_start(x[:], src_ap[ti * 128:(ti + 1) * 128, :])
                sq = sqp.tile([128, DF], dt.bfloat16, tag="sq", name="sq")
                norm2 = nrm.tile([128, 1], dt.float32, tag="norm2", name="norm2")
                nc.scalar.activation(sq[:], x[:], AF.Square, accum_out=norm2[:])
                nrm_v = nrm.tile([128, 1], dt.float32, tag="norm", name="norm")
                nc.scalar.activation(nrm_v[:], norm2[:], AF.Sqrt)
                nc.vector.tensor_scalar_max(nrm_v[:], nrm_v[:], 1e-8)
                rn = nrm.tile([128, 1], dt.float32, tag="rnorm", name="rnorm")
                nc.vector.reciprocal(rn[:], nrm_v[:])
                xn = xnp.tile([128, DF], dt.bfloat16, tag="xn", name="xn")
                nc.vector.tensor_scalar_mul(xn[:], x[:], rn[:])
                for t in range(2):
                    ps = tpp.tile([128, 128], dt.bfloat16, tag="tp", name="tp")
                    nc.tensor.transpose(ps[:], xn[:, t * 128:(t + 1) * 128], ident_sb[:])
                    eng = nc.vector if t == 0 else nc.scalar
                    if t == 0:
                        nc.vector.tensor_copy(dstT[t][:, ti * 128:(ti + 1) * 128], ps[:])
                    else:
                        nc.scalar.copy(dstT[t][:, ti * 128:(ti + 1) * 128], ps[:])

        norm_tiles(logits_d, N // 128, xnT)
        norm_tiles(mine_d, NB, mnT)

        # Per-segment column sums of xn^T -> G [feat, n_segs]; gvec_g = ||G_g||^2
        for t in range(2):
            for g in range(n_segs):
                nc.vector.tensor_reduce(
                    G[t][:, g:g + 1],
                    xnT[t][:, seg_off[g]:seg_off[g] + seg_w[g]],
                    axis=X, op=ALU.add,
                )
            nc.vector.tensor_tensor(G2[t][:], G[t][:], G[t][:], ALU.mult)
        psg = gp.tile([n_segs, 1], dt.float32, tag="gps", name="gps")
        for t in range(2):
            nc.tensor.matmul(psg[:], G2[t][:], ones_t[:], start=(t == 0), stop=(t == 1))
        nc.vector.tensor_copy(gsb[:], psg[:])
        nc.sync.dma_start(gvec_d[:], gsb[:])

        for b in range(NB):
            bsb = bp.tile([128, 2], dt.float32, tag="bnd", name="bnd")
            nc.sync.dma_start(bsb[:], bounds_d[b * 128:(b + 1) * 128, :])
            win = WIN[b]
            e_strip = ep.tile([128, N], dt.bfloat16, tag="e", name="e")
            rs = rsp.tile([128, NCH], dt.float32, tag="rs", name="rs")
            for c in range(NCH):
                ps = mmp.tile([128, CH], dt.float32, tag="mm", name="mm")
                for t in range(2):
                    nc.tensor.matmul(
                        ps[:],
                        mnT[t][:, b * 128:(b + 1) * 128],
                        xnT[t][:, c * CH:(c + 1) * CH],
                        start=(t == 0), stop=(t == 1),
                    )
                nc.scalar.activation(
                    e_strip[:, c * CH:(c + 1) * CH], ps[:], AF.Exp,
                    scale=T_SCALE, accum_out=rs[:, c:c + 1],
                )
            rsum = sm.tile([128, 1], dt.float32, tag="rsum", name="rsum")
            nc.vector.tensor_reduce(rsum[:], rs[:], axis=X, op=ALU.add)
            # mask[r, j] = (iota_j >= st_r) * (iota_j < en_r)
            m1 = mkp.tile([128, WF], dt.bfloat16, tag="m1", name="m1")
            nc.vector.tensor_scalar(
                m1[:], iota_t[:], bsb[:, 0:1], None, ALU.is_ge,
            )
            mask = mkp.tile([128, WF], dt.bfloat16, tag="mask", name="mask")
            nc.vector.scalar_tensor_tensor(
                mask[:], iota_t[:], bsb[:, 1:2], m1[:], ALU.is_lt, ALU.mult,
            )
            junk = jkp.tile([128, WF], dt.bfloat16, tag="junk", name="junk")
            ssum = sm.tile([128, 1], dt.float32, tag="ssum", name="ssum")
            nc.vector.scalar_tensor_tensor(
                junk[:], e_strip[:, win:win + WF], 1.0, mask[:],
                ALU.mult, ALU.mult, accum_out=ssum[:],
            )
            Dv = sm.tile([128, 1], dt.float32, tag="Dv", name="Dv")
            nc.vector.tensor_tensor(Dv[:], rsum[:], ssum[:], ALU.subtract)
            lg = lgp.tile([128, WF], dt.float32, tag="lg", name="lg")
            nc.scalar.activation(lg[:], e_strip[:, win:win + WF], AF.Ln, bias=Dv[:])
            lgrow = sm.tile([128, 1], dt.float32, tag="lgrow", name="lgrow")
            nc.vector.scalar_tensor_tensor(
                junk[:], lg[:], 1.0, mask[:], ALU.mult, ALU.mult, accum_out=lgrow[:],
            )
            corr = sm.tile([128, 1], dt.float32, tag="corr", name="corr")
            nc.scalar.activation(corr[:], Dv[:], AF.Ln, bias=e2c[:])
            tmp = sm.tile([128, 1], dt.float32, tag="tmp", name="tmp")
            nc.vector.scalar_tensor_tensor(
                tmp[:], lgrow[:], 1.0, corr[:], ALU.mult, ALU.subtract,
            )
            nc.vector.tensor_tensor(acc_t[:], acc_t[:], tmp[:], ALU.add)
        nc.sync.dma_start(acc_d[:], acc_t[:])


def _prep(logits, label):
    logits = np.asarray(logits, dtype=np.float32)
    lab = np.asarray(label).ravel()
    assert logits.shape == (N, DF), logits.shape
    perm = np.argsort(lab, kind="stable")
    slog = np.ascontiguousarray(logits[perm])
    labs = lab[perm]
    uniq, counts = np.unique(labs, return_counts=True)
    seg_off = np.concatenate([[0], np.cumsum(counts)[:-1]]).astype(np.int64)
    seg_end = seg_off + counts
    seg_idx = np.searchsorted(uniq, labs)
    row_st = seg_off[seg_idx]
    row_en = seg_end[seg_idx]

    # Slot b (b = 0..NB-1) is executed at the same program point on every
    # core; core c's slot-b block is global block c + NCORES*b. Slot b thus
    # spans the consecutive global blocks [NCORES*b, NCORES*b + NCORES) =
    # rows [1024b, 1024(b+1)), whose label-segment windows are adjacent
    # (rows sorted by label), so one baked chunk-aligned window per slot
    # covers all cores.
    grp = N // NB                       # rows per slot group (1024)
    mn = row_st.reshape(NB, grp).min(axis=1)
    mx = row_en.reshape(NB, grp).max(axis=1)
    w0 = (mn // CH) * CH
    W_CH = max(1, int(np.ceil((mx - w0).max() / CH)))
    WF = W_CH * CH
    wins = np.minimum(w0, N - WF)
    assert (mx <= wins + WF).all() and (mn >= wins).all() and (wins >= 0).all()

    win_of_row = np.repeat(wins, grp)
    st_rel = (row_st - win_of_row).astype(np.float32)
    en_rel = (row_en - win_of_row).astype(np.float32)
    bounds = np.stack([st_rel, en_rel], axis=1)  # [N, 2] f32, exact ints
    return slog, bounds, wins.astype(np.int64), W_CH, seg_off, counts.astype(np.int64)


def kernel(logits, label):
    global LAST_EXEC_NS, LAST_RESULTS
    slog, bounds, wins, W_CH, seg_off, seg_w = _prep(logits, label)

    import concourse.bacc as bacc
    from concourse.bass_utils import run_bass_kernel_spmd

    nc = bacc.Bacc("TRN2", target_bir_lowering=False, debug=False)
    _emit(nc, W_CH, [int(w) for w in wins],
          [int(o) for o in seg_off], [int(w) for w in seg_w])
    nc.compile()

    ident = np.eye(128, dtype=ml_dtypes.bfloat16)
    in_maps = []
    for c in range(NCORES):
        rows = np.concatenate([
            np.arange((c + NCORES * b) * 128, (c + NCORES * b) * 128 + 128)
            for b in range(NB)
        ])
        in_maps.append({
            "logits": slog,
            "mine": np.ascontiguousarray(slog[rows]),
            "bounds": np.ascontiguousarray(bounds[rows]),
            "ident": ident,
        })

    kwargs = {}
    if TRACE:
        _enable_ntff_hook()
        kwargs["trace"] = True
    res = run_bass_kernel_spmd(nc, in_maps, core_ids=list(range(NCORES)), **kwargs)
    LAST_RESULTS = res
    if TRACE:
        LAST_EXEC_NS = res.exec_time_ns

    total = sum(
        res.results[c]["acc"].astype(np.float64).sum() for c in range(NCORES)
    )
    gsum = res.results[0]["gvec"].astype(np.float64).sum()
    loss = (total - 2.0 * (gsum - N)) / (2.0 * N)
    return np.float32(loss)


def _enable_ntff_hook():
    import types
    import concourse.bass_utils as bass_utils

    if "antenv.axon_hooks" not in sys.modules:
        mod = types.ModuleType("antenv.axon_hooks")
        mod._hook = None
        mod.set_axon_ntff_profile_hook = lambda h: setattr(mod, "_hook", h)
        mod.get_axon_ntff_profile_hook = lambda: mod._hook
        sys.modules["antenv.axon_hooks"] = mod
    from antenv.axon_hooks import set_axon_ntff_profile_hook, get_axon_ntff_profile_hook
    if get_axon_ntff_profile_hook() is None:
        from trn_agent_boot.trn_boot import _ntff_profile_via_ctypes
        set_axon_ntff_profile_hook(_ntff_profile_via_ctypes("/opt/axon/libaxon_pjrt.so"))
    bass_utils.upload_artifacts = lambda tmpdir: tmpdir
